# revision 25
# baseline (speedup 1.0000x reference)
"""Trainium2 Bass kernel for multi-head attention (B=4, N=2048, DIM=1024, H=16, DH=64).

Sharding: 8 cores = 4 batches x 2 head-groups (8 heads each): column-parallel
qkv, row-parallel out-proj.  Each core computes q/k/v for its 8 heads over the
full 2048 queries, attention per head-pair with PE tile packing, and a PARTIAL
output projection; the host gather sums the two partial outputs per batch and
adds the bias.

PE schedule: software-pipelined units. Unit (s,c) interleaves, in groups of 3
j-tiles, the scores matmuls of (s,c) [(64,128) row-tiled T0/T8 pairs, 2x
concurrent] with the AV + denominator matmuls of the previous unit [(128,64)
col-tiled T0/T1, 2x concurrent; denominators via ones[128,64] matmul give a
64-row broadcast for free].  Softmax exp runs on ScalarE for 12/16 j-tiles and
on the DVE for 4/16 via a 2-pass bit-trick exp (floor extract + quadratic
mantissa correction, ~0.3% shape error, uniform scale cancels in softmax).
Normalize = reciprocal_approx_fast + one tensor_mul.  QKV projection chunks
ride as PE fillers inside units; out-proj runs as the epilogue.
"""

import numpy as np
import ml_dtypes

import concourse.bass as bass
import concourse.tile as tile
from concourse import bacc, mybir
from concourse import bass_utils

B, N, DIM = 4, 2048, 1024
HEADS, DH = 16, 64
INNER = HEADS * DH
SCALE = DH ** -0.5
NCORES = 8
HG = 8                    # heads per core
NP = HG // 2              # head pairs per core
KT = DIM // 128           # contraction tiles for projections
NT = N // 128             # 16 j tiles
NC = N // 512             # 4 i chunks
BF16 = mybir.dt.bfloat16
F32 = mybir.dt.float32
I16 = mybir.dt.int16

EXPA = float(np.float32(SCALE / np.log(2)))   # y = score * EXPA = log2(e^(s*SCALE))
EXPD = -0.34                                  # quadratic mantissa correction coeff
DVESET = (4, 7, 10, 13)                       # j-tiles exp'd on DVE (bit-trick)

_CACHE = {}


def _make_exp16():
    """Register the 2nd-pass exp op (quadratic-corrected Schraudolph ->
    bf16 bits) in the custom-DVE registry, reusing a spare opcode row."""
    from concourse.dve_spec import (
        Spec, Src0, Src1, C0, C1, C2, One, lower, _has_src1)
    from concourse.dve_uop import DveOpSpec
    from concourse import dve_ops
    from concourse.dve_ops import DveOp, get_dve_sub_opcode

    name = "CODY_WAITE_CASCADE"
    y = Src0 * C0
    f = (y - Src1) + (C2 - One)
    m = f * ((One + C1) - C1 * f)
    body = (Src1 + m) * C2

    def ref(in0, in1, s0, s1, imm2):
        yy = in0.astype(np.float32) * s0
        ff = (yy - in1) + (imm2 - 1.0)
        mm = ff * ((1.0 + s1) - s1 * ff)
        return (in1 + mm) * imm2

    spec = Spec(body=body, reference=ref)
    shas = {}
    for ver in ("v3", "v4"):
        uops = lower(spec, ver=ver)
        shas[ver] = DveOpSpec(name=name, opcode=get_dve_sub_opcode(name),
                              uops=uops, rd1_en=_has_src1(spec)).sha(ver)
    op = DveOp(name, spec, subdim=False, uops_sha=shas)
    dve_ops.OPS.append(op)
    return op


EXP16 = _make_exp16()


def _build_program():
    nc = bacc.Bacc("TRN2", target_bir_lowering=False, debug=False)
    xT_d = nc.dram_tensor("xT", [DIM, N], BF16, kind="ExternalInput")
    wq_d = nc.dram_tensor("wq", [DIM, 512], BF16, kind="ExternalInput")
    wk_d = nc.dram_tensor("wk", [DIM, 512], BF16, kind="ExternalInput")
    wv_d = nc.dram_tensor("wv", [DIM, 512], BF16, kind="ExternalInput")
    wo_d = nc.dram_tensor("wo", [512, DIM], BF16, kind="ExternalInput")
    out_d = nc.dram_tensor("out", [N, DIM], BF16, kind="ExternalOutput")
    with tile.TileContext(nc) as tc:
        _emit(tc, nc, xT_d, wq_d, wk_d, wv_d, wo_d, out_d)
    nc.compile()
    return nc


def _emit(tc, nc, xT_d, wq_d, wk_d, wv_d, wo_d, out_d):
    from contextlib import ExitStack

    xT_r = xT_d.ap().rearrange("(t p) n -> p t n", p=128)    # [128, 8, 2048]
    wq_r = wq_d.ap().rearrange("(t p) e -> p t e", p=128)    # [128, 8, 512]
    wk_r = wk_d.ap().rearrange("(t p) e -> p t e", p=128)
    wv_r = wv_d.ap().rearrange("(t p) e -> p t e", p=128)
    wo_r = wo_d.ap().rearrange("(t p) d -> p t d", p=128)    # [128, 4, 1024]

    with ExitStack() as ctx:
        consts = ctx.enter_context(tc.tile_pool(name="consts", bufs=1))
        qkv = ctx.enter_context(tc.tile_pool(name="qkv", bufs=1))
        atp = ctx.enter_context(tc.tile_pool(name="atp", bufs=16))
        up = ctx.enter_context(tc.tile_pool(name="up", bufs=4))
        recp = ctx.enter_context(tc.tile_pool(name="recp", bufs=2))
        outp = ctx.enter_context(tc.tile_pool(name="outp", bufs=2))
        xp = ctx.enter_context(tc.tile_pool(name="xp", bufs=1))
        wp = ctx.enter_context(tc.tile_pool(name="wp", bufs=1))
        wqk = ctx.enter_context(tc.tile_pool(name="wqk", bufs=4))
        psc = ctx.enter_context(tc.tile_pool(name="psc", bufs=3, space="PSUM"))
        pwk = ctx.enter_context(tc.tile_pool(name="pwk", bufs=2, space="PSUM"))

        # ---- input DMAs, spread over 3 queues; first-needed first ----
        wqs = {0: wqk.tile([128, KT, 128], BF16, tag="w", name="wq0")}
        wks = {0: wqk.tile([128, KT, 128], BF16, tag="w", name="wk0")}
        nc.sync.dma_start(out=wqs[0], in_=wq_r[:, :, 0:128])
        nc.scalar.dma_start(out=wks[0], in_=wk_r[:, :, 0:128])
        xTk = xp.tile([128, KT, N], BF16)
        qs = [nc.sync, nc.gpsimd, nc.scalar]
        for k in range(KT):
            qs[k % 3].dma_start(out=xTk[:, k, :], in_=xT_r[:, k, :])
        wv_sb = wp.tile([128, KT, 512], BF16)
        nc.gpsimd.dma_start(out=wv_sb, in_=wv_r)
        wo_sb = consts.tile([128, 4, DIM], BF16)
        nc.gpsimd.dma_start(out=wo_sb, in_=wo_r)
        ones = consts.tile([128, 64], BF16)
        nc.vector.memset(ones, 1.0)

        # ---- persistent sbuf ----
        qT = qkv.tile([128, NP, N], BF16)     # pair s rows: head 2s | 2s+1
        kT = qkv.tile([128, NP, N], BF16)
        v = qkv.tile([128, NT, HG, DH], BF16)
        aoT = qkv.tile([128, NP, N], BF16)

        # ---- emitters ----
        def w_fetch(s):
            wqs[s] = wqk.tile([128, KT, 128], BF16, tag="w", name=f"wq{s}")
            wks[s] = wqk.tile([128, KT, 128], BF16, tag="w", name=f"wk{s}")
            nc.sync.dma_start(out=wqs[s], in_=wq_r[:, :, 128 * s:128 * (s + 1)])
            nc.sync.dma_start(out=wks[s], in_=wk_r[:, :, 128 * s:128 * (s + 1)])

        def proj_tile(dst, w_sb, s, ih):
            # [128 e, 1024 i] chunk of q or k pair-slice s
            ps = psc.tile([128, 1024], F32, tag="sc", name=f"pj{dst is kT}_{s}_{ih}")
            for half in range(2):
                sl = slice(512 * half, 512 * (half + 1))
                for k in range(KT):
                    nc.tensor.matmul(
                        ps[:, sl], w_sb[:, k, :],
                        xTk[:, k, 1024 * ih + 512 * half:
                            1024 * ih + 512 * (half + 1)],
                        start=(k == 0), stop=(k == KT - 1))
            nc.vector.tensor_copy(
                out=dst[:, s, 1024 * ih:1024 * (ih + 1)], in_=ps)

        def v_tile(j):
            # v j-tile pair (2j, 2j+1) for all 8 heads
            ps = psc.tile([128, 1024], F32, tag="sc", name=f"v{j}")
            for tt in range(2):
                for k in range(KT):
                    nc.tensor.matmul(
                        ps[:, 512 * tt:512 * (tt + 1)],
                        xTk[:, k, 128 * (2 * j + tt):128 * (2 * j + tt + 1)],
                        wv_sb[:, k, :], start=(k == 0), stop=(k == KT - 1))
            nc.vector.tensor_copy(
                out=v[:, 2 * j:2 * j + 2, :, :],
                in_=ps.rearrange("p (t2 h d) -> p t2 h d", t2=2, h=HG))

        ats = {}

        def sc_tile(s, c, t):
            # scores^T pair s, i-chunk c, j-tile t + exp dispatch
            isl = slice(512 * c, 512 * (c + 1))
            sc = psc.tile([128, 1024], F32, tag="sc", name=f"sc{s}_{c}_{t}")
            nc.tensor.matmul(
                sc[:, 0:512], kT[0:64, s, 128 * t:128 * (t + 1)],
                qT[0:64, s, isl], start=True, stop=True)
            nc.tensor.matmul(
                sc[:, 512:1024], kT[64:128, s, 128 * t:128 * (t + 1)],
                qT[64:128, s, isl], start=True, stop=True)
            at = atp.tile([128, 1024], BF16, tag="at", name=f"at{s}_{c}_{t}")
            ats[(s, c, t)] = at
            if t in DVESET:
                u = up.tile([128, 1024], I16, tag="u", name=f"u{s}_{c}_{t}")
                for hf in range(2):
                    sl = slice(512 * hf, 512 * (hf + 1))
                    nc.vector.tensor_scalar(
                        out=u[:, sl], in0=sc[:, sl], scalar1=EXPA,
                        scalar2=126.5, op0=mybir.AluOpType.mult,
                        op1=mybir.AluOpType.add)
                    nc.vector._custom_dve(
                        EXP16, out=at.bitcast(I16)[:, sl], in0=sc[:, sl],
                        in1=u[:, sl], s0=EXPA, s1=EXPD, imm2=128.0)
            else:
                nc.scalar.activation(
                    out=at, in_=sc,
                    func=mybir.ActivationFunctionType.Exp, scale=SCALE)

        def avden_alloc(s, c):
            av = pwk.tile([128, 512], F32, tag="wk", name=f"av{s}_{c}")
            den = pwk.tile([128, 512], F32, tag="wk", name=f"den{s}_{c}")
            return av, den

        def avden_tile(avden, s, c, t):
            av, den = avden
            at = ats.pop((s, c, t))
            st, sp = (t == 0), (t == NT - 1)
            a0 = at[:, 0:512].bitcast(BF16) if t in DVESET else at[:, 0:512]
            a1 = at[:, 512:1024].bitcast(BF16) if t in DVESET else at[:, 512:1024]
            nc.tensor.matmul(av[0:64, :], v[:, t, 2 * s, :], a0,
                             start=st, stop=sp)
            nc.tensor.matmul(av[64:128, :], v[:, t, 2 * s + 1, :], a1,
                             start=st, stop=sp)
            nc.tensor.matmul(den[0:64, :], ones, a0, start=st, stop=sp)
            nc.tensor.matmul(den[64:128, :], ones, a1, start=st, stop=sp)

        def normalize(avden, s, c):
            av, den = avden
            rec = recp.tile([128, 512], F32, tag="rec", name=f"rec{s}_{c}")
            nc.vector.reciprocal_approx_fast(out=rec, in_=den)
            nc.vector.tensor_mul(
                out=aoT[:, s, 512 * c:512 * (c + 1)], in0=av, in1=rec)

        def po_tile(it):
            po = psc.tile([128, 1024], F32, tag="sc", name=f"po{it}")
            for half in range(2):
                sl = slice(512 * half, 512 * (half + 1))
                for s in range(NP):
                    nc.tensor.matmul(
                        po[:, sl], aoT[:, s, 128 * it:128 * (it + 1)],
                        wo_sb[:, s, sl], start=(s == 0), stop=(s == NP - 1))
            ot = outp.tile([128, DIM], BF16, tag="out", name=f"ot{it}")
            nc.scalar.copy(out=ot, in_=po)
            nc.sync.dma_start(
                out=out_d.ap()[128 * it:128 * (it + 1), :], in_=ot)

        def unit(cur, prev, fillers):
            # interleave: scores of `cur` with AV/den of `prev`, 3-t groups;
            # normalize(prev) goes right after its last AV group so the DVE
            # reaches it before the next unit's first AV matmul needs the bank
            avden = avden_alloc(*prev) if prev else None
            fi = 0
            for g in range(0, NT, 3):
                if fillers and g in (3, 6, 9, 12) and fi < len(fillers):
                    fillers[fi]()
                    fi += 1
                for t in range(g, min(g + 3, NT)):
                    if prev:
                        avden_tile(avden, prev[0], prev[1], t)
                if prev and g + 3 >= NT:
                    normalize(avden, *prev)
                for t in range(g, min(g + 3, NT)):
                    if cur is not None:
                        sc_tile(cur[0], cur[1], t)
            while fi < len(fillers):
                fillers[fi]()
                fi += 1

        # ---- schedule: pair-major, software-pipelined by one unit ----
        proj_tile(qT, wqs[0], 0, 0)
        proj_tile(qT, wqs[0], 0, 1)
        w_fetch(1)
        proj_tile(kT, wks[0], 0, 0)
        proj_tile(kT, wks[0], 0, 1)

        seq = [(s, c) for s in range(NP) for c in range(NC)]
        prev = None
        vleft = list(range(NT // 2))
        for idx, cur in enumerate(seq):
            s, c = cur
            fillers = []
            if idx == 0:
                fillers = [(lambda j=j: v_tile(j)) for j in vleft[0:5]]
            elif idx == 1:
                fillers = [(lambda j=j: v_tile(j)) for j in vleft[5:8]]
            elif c == 2 and s < NP - 1:
                sn = s + 1
                if sn not in wqs:
                    w_fetch(sn)
                fillers = [lambda sn=sn: proj_tile(qT, wqs[sn], sn, 0),
                           lambda sn=sn: proj_tile(qT, wqs[sn], sn, 1)]
            elif c == 3 and s < NP - 1:
                sn = s + 1
                fillers = [lambda sn=sn: proj_tile(kT, wks[sn], sn, 0),
                           lambda sn=sn: proj_tile(kT, wks[sn], sn, 1)]
            elif idx >= 14:
                # out-proj chunk c-2 is complete once unit #idx-1 finished
                # (aoT(3, c-2) normalized inside unit #idx-1)
                pc = idx - 14
                fillers = [(lambda it=it: po_tile(it))
                           for it in range(4 * pc, 4 * pc + 4)]
            unit(cur, prev, fillers)
            prev = cur
        # drain av of the last unit, with out-proj chunk 2 as filler
        unit(None, prev, [(lambda it=it: po_tile(it)) for it in range(8, 12)])
        for it in range(12, 16):      # epilogue: last out-proj chunk
            po_tile(it)


def get_program():
    if "nc" not in _CACHE:
        _CACHE["nc"] = _build_program()
    return _CACHE["nc"]


def make_in_maps(x, w_qkv, w_out, b_out):
    bf = ml_dtypes.bfloat16
    w = np.ascontiguousarray(w_qkv, np.float32)
    wo = np.ascontiguousarray(w_out, np.float32)
    xTs = [np.ascontiguousarray(np.asarray(x[bb], np.float32).T).astype(bf)
           for bb in range(B)]
    in_maps = []
    for core in range(NCORES):
        bb, hg = core // 2, core % 2
        cs = slice(512 * hg, 512 * (hg + 1))
        in_maps.append({
            "xT": xTs[bb],
            "wq": np.ascontiguousarray(w[:, 0:1024][:, cs]).astype(bf),
            "wk": np.ascontiguousarray(w[:, 1024:2048][:, cs]).astype(bf),
            "wv": np.ascontiguousarray(w[:, 2048:3072][:, cs]).astype(bf),
            "wo": np.ascontiguousarray(wo[cs, :]).astype(bf),
        })
    return in_maps


def kernel(x, w_qkv, w_out, b_out):
    nc = get_program()
    in_maps = make_in_maps(x, w_qkv, w_out, b_out)
    res = bass_utils.run_bass_kernel_spmd(nc, in_maps, core_ids=list(range(NCORES)))
    bias = np.asarray(b_out, np.float32)[None, :]
    out = np.empty((B, N, DIM), np.float32)
    for bb in range(B):
        out[bb] = (res.results[2 * bb]["out"].astype(np.float32)
                   + res.results[2 * bb + 1]["out"].astype(np.float32) + bias)
    return out


# revision 26
# speedup vs baseline: 1.0890x; 1.0890x over previous
"""Trainium2 Bass kernel for multi-head attention (B=4, N=2048, DIM=1024, H=16, DH=64).

Sharding: 8 cores = 4 batches x 2 head-groups (8 heads each): column-parallel
qkv, row-parallel out-proj.  Each core computes q/k/v for its 8 heads over the
full 2048 queries, attention per head-pair with PE tile packing, and a PARTIAL
output projection; the host gather sums the two partial outputs per batch and
adds the bias.

PE schedule: software-pipelined units. Unit (s,c) interleaves, in groups of 3
j-tiles, the scores matmuls of (s,c) [(64,128) row-tiled T0/T8 pairs, 2x
concurrent] with the AV + denominator matmuls of the previous unit [(128,64)
col-tiled T0/T1, 2x concurrent; denominators via ones[128,64] matmul give a
64-row broadcast for free].  Softmax exp runs on ScalarE for 12/16 j-tiles and
on the DVE for 4/16 via a 2-pass bit-trick exp (floor extract + quadratic
mantissa correction, ~0.3% shape error, uniform scale cancels in softmax).
Normalize = reciprocal_approx_fast + one tensor_mul.  QKV projection chunks
ride as PE fillers inside units; out-proj runs as the epilogue.
"""

import numpy as np
import ml_dtypes

import concourse.bass as bass
import concourse.tile as tile
from concourse import bacc, mybir
from concourse import bass_utils

B, N, DIM = 4, 2048, 1024
HEADS, DH = 16, 64
INNER = HEADS * DH
SCALE = DH ** -0.5
NCORES = 8
HG = 8                    # heads per core
NP = HG // 2              # head pairs per core
KT = DIM // 128           # contraction tiles for projections
NT = N // 128             # 16 j tiles
NC = N // 512             # 4 i chunks
BF16 = mybir.dt.bfloat16
F32 = mybir.dt.float32
I16 = mybir.dt.int16

EXPA = float(np.float32(SCALE / np.log(2)))   # y = score * EXPA = log2(e^(s*SCALE))
EXPD = -0.34                                  # quadratic mantissa correction coeff
DVESET = (4, 7, 10, 13)                       # j-tiles exp'd on DVE (bit-trick)

_CACHE = {}


def _make_exp16():
    """Register the 2nd-pass exp op (quadratic-corrected Schraudolph ->
    bf16 bits) in the custom-DVE registry, reusing a spare opcode row."""
    from concourse.dve_spec import (
        Spec, Src0, Src1, C0, C1, C2, One, lower, _has_src1)
    from concourse.dve_uop import DveOpSpec
    from concourse import dve_ops
    from concourse.dve_ops import DveOp, get_dve_sub_opcode

    name = "CODY_WAITE_CASCADE"
    y = Src0 * C0
    f = (y - Src1) + (C2 - One)
    m = f * ((One + C1) - C1 * f)
    body = (Src1 + m) * C2

    def ref(in0, in1, s0, s1, imm2):
        yy = in0.astype(np.float32) * s0
        ff = (yy - in1) + (imm2 - 1.0)
        mm = ff * ((1.0 + s1) - s1 * ff)
        return (in1 + mm) * imm2

    spec = Spec(body=body, reference=ref)
    shas = {}
    for ver in ("v3", "v4"):
        uops = lower(spec, ver=ver)
        shas[ver] = DveOpSpec(name=name, opcode=get_dve_sub_opcode(name),
                              uops=uops, rd1_en=_has_src1(spec)).sha(ver)
    op = DveOp(name, spec, subdim=False, uops_sha=shas)
    dve_ops.OPS.append(op)
    return op


EXP16 = _make_exp16()


def _build_program():
    nc = bacc.Bacc("TRN2", target_bir_lowering=False, debug=False)
    xT_d = nc.dram_tensor("xT", [DIM, N], BF16, kind="ExternalInput")
    wq_d = nc.dram_tensor("wq", [DIM, 512], BF16, kind="ExternalInput")
    wk_d = nc.dram_tensor("wk", [DIM, 512], BF16, kind="ExternalInput")
    wv_d = nc.dram_tensor("wv", [DIM, 512], BF16, kind="ExternalInput")
    wo_d = nc.dram_tensor("wo", [512, DIM], BF16, kind="ExternalInput")
    out_d = nc.dram_tensor("out", [N, DIM], BF16, kind="ExternalOutput")
    with tile.TileContext(nc) as tc:
        _emit(tc, nc, xT_d, wq_d, wk_d, wv_d, wo_d, out_d)
    nc.compile()
    return nc


def _emit(tc, nc, xT_d, wq_d, wk_d, wv_d, wo_d, out_d):
    from contextlib import ExitStack

    xT_r = xT_d.ap().rearrange("(t p) n -> p t n", p=128)    # [128, 8, 2048]
    wq_r = wq_d.ap().rearrange("(t p) e -> p t e", p=128)    # [128, 8, 512]
    wk_r = wk_d.ap().rearrange("(t p) e -> p t e", p=128)
    wv_r = wv_d.ap().rearrange("(t p) e -> p t e", p=128)
    wo_r = wo_d.ap().rearrange("(t p) d -> p t d", p=128)    # [128, 4, 1024]

    with ExitStack() as ctx:
        consts = ctx.enter_context(tc.tile_pool(name="consts", bufs=1))
        qkv = ctx.enter_context(tc.tile_pool(name="qkv", bufs=1))
        atp = ctx.enter_context(tc.tile_pool(name="atp", bufs=16))
        up = ctx.enter_context(tc.tile_pool(name="up", bufs=4))
        yp = ctx.enter_context(tc.tile_pool(name="yp", bufs=3))
        recp = ctx.enter_context(tc.tile_pool(name="recp", bufs=2))
        outp = ctx.enter_context(tc.tile_pool(name="outp", bufs=2))
        xp = ctx.enter_context(tc.tile_pool(name="xp", bufs=1))
        wp = ctx.enter_context(tc.tile_pool(name="wp", bufs=1))
        wqk = ctx.enter_context(tc.tile_pool(name="wqk", bufs=4))
        psc = ctx.enter_context(tc.tile_pool(name="psc", bufs=3, space="PSUM"))
        pwk = ctx.enter_context(tc.tile_pool(name="pwk", bufs=2, space="PSUM"))

        # ---- input DMAs, spread over 3 queues; first-needed first ----
        wqs = {0: wqk.tile([128, KT, 128], BF16, tag="w", name="wq0")}
        wks = {0: wqk.tile([128, KT, 128], BF16, tag="w", name="wk0")}
        nc.sync.dma_start(out=wqs[0], in_=wq_r[:, :, 0:128])
        nc.scalar.dma_start(out=wks[0], in_=wk_r[:, :, 0:128])
        xTk = xp.tile([128, KT, N], BF16)
        qs = [nc.sync, nc.gpsimd, nc.scalar]
        for k in range(KT):
            qs[k % 3].dma_start(out=xTk[:, k, :], in_=xT_r[:, k, :])
        wv_sb = wp.tile([128, KT, 512], BF16)
        nc.gpsimd.dma_start(out=wv_sb, in_=wv_r)
        wo_sb = consts.tile([128, 4, DIM], BF16)
        nc.gpsimd.dma_start(out=wo_sb, in_=wo_r)
        ones = consts.tile([128, 64], BF16)
        nc.vector.memset(ones, 1.0)

        # ---- persistent sbuf ----
        qT = qkv.tile([128, NP, N], BF16)     # pair s rows: head 2s | 2s+1
        kT = qkv.tile([128, NP, N], BF16)
        v = qkv.tile([128, NT, HG, DH], BF16)
        aoT = qkv.tile([128, NP, N], BF16)

        # ---- emitters ----
        def w_fetch(s):
            wqs[s] = wqk.tile([128, KT, 128], BF16, tag="w", name=f"wq{s}")
            wks[s] = wqk.tile([128, KT, 128], BF16, tag="w", name=f"wk{s}")
            nc.sync.dma_start(out=wqs[s], in_=wq_r[:, :, 128 * s:128 * (s + 1)])
            nc.sync.dma_start(out=wks[s], in_=wk_r[:, :, 128 * s:128 * (s + 1)])

        def proj_tile(dst, w_sb, s, ih):
            # [128 e, 1024 i] chunk of q or k pair-slice s
            ps = psc.tile([128, 1024], F32, tag="sc", name=f"pj{dst is kT}_{s}_{ih}")
            for half in range(2):
                sl = slice(512 * half, 512 * (half + 1))
                for k in range(KT):
                    nc.tensor.matmul(
                        ps[:, sl], w_sb[:, k, :],
                        xTk[:, k, 1024 * ih + 512 * half:
                            1024 * ih + 512 * (half + 1)],
                        start=(k == 0), stop=(k == KT - 1))
            nc.vector.tensor_copy(
                out=dst[:, s, 1024 * ih:1024 * (ih + 1)], in_=ps)

        def v_tile(j):
            # v j-tile pair (2j, 2j+1) for all 8 heads
            ps = psc.tile([128, 1024], F32, tag="sc", name=f"v{j}")
            for tt in range(2):
                for k in range(KT):
                    nc.tensor.matmul(
                        ps[:, 512 * tt:512 * (tt + 1)],
                        xTk[:, k, 128 * (2 * j + tt):128 * (2 * j + tt + 1)],
                        wv_sb[:, k, :], start=(k == 0), stop=(k == KT - 1))
            nc.vector.tensor_copy(
                out=v[:, 2 * j:2 * j + 2, :, :],
                in_=ps.rearrange("p (t2 h d) -> p t2 h d", t2=2, h=HG))

        ats = {}

        def sc_tile(s, c, t):
            # scores^T pair s, i-chunk c, j-tile t + exp dispatch
            isl = slice(512 * c, 512 * (c + 1))
            sc = psc.tile([128, 1024], F32, tag="sc", name=f"sc{s}_{c}_{t}")
            nc.tensor.matmul(
                sc[:, 0:512], kT[0:64, s, 128 * t:128 * (t + 1)],
                qT[0:64, s, isl], start=True, stop=True)
            nc.tensor.matmul(
                sc[:, 512:1024], kT[64:128, s, 128 * t:128 * (t + 1)],
                qT[64:128, s, isl], start=True, stop=True)
            at = atp.tile([128, 1024], BF16, tag="at", name=f"at{s}_{c}_{t}")
            ats[(s, c, t)] = at
            if t in DVESET:
                ysb = yp.tile([128, 1024], F32, tag="y", name=f"y{s}_{c}_{t}")
                for hf in range(2):
                    sl = slice(512 * hf, 512 * (hf + 1))
                    nc.vector.tensor_scalar(
                        out=ysb[:, sl], in0=sc[:, sl], scalar1=EXPA,
                        scalar2=0.0, op0=mybir.AluOpType.mult,
                        op1=mybir.AluOpType.add)
                u = up.tile([128, 1024], I16, tag="u", name=f"u{s}_{c}_{t}")
                nc.vector.tensor_scalar(
                    out=u, in0=ysb, scalar1=1.0, scalar2=126.5,
                    op0=mybir.AluOpType.mult, op1=mybir.AluOpType.add)
                nc.vector._custom_dve(EXP16, out=at.bitcast(I16), in0=ysb,
                                      in1=u, s0=1.0, s1=EXPD, imm2=128.0)
            else:
                nc.scalar.activation(
                    out=at, in_=sc,
                    func=mybir.ActivationFunctionType.Exp, scale=SCALE)

        def avden_alloc(s, c):
            av = pwk.tile([128, 512], F32, tag="wk", name=f"av{s}_{c}")
            den = pwk.tile([128, 512], F32, tag="wk", name=f"den{s}_{c}")
            return av, den

        def avden_tile(avden, s, c, t):
            av, den = avden
            at = ats.pop((s, c, t))
            st, sp = (t == 0), (t == NT - 1)
            a0 = at[:, 0:512].bitcast(BF16) if t in DVESET else at[:, 0:512]
            a1 = at[:, 512:1024].bitcast(BF16) if t in DVESET else at[:, 512:1024]
            nc.tensor.matmul(av[0:64, :], v[:, t, 2 * s, :], a0,
                             start=st, stop=sp)
            nc.tensor.matmul(av[64:128, :], v[:, t, 2 * s + 1, :], a1,
                             start=st, stop=sp)
            nc.tensor.matmul(den[0:64, :], ones, a0, start=st, stop=sp)
            nc.tensor.matmul(den[64:128, :], ones, a1, start=st, stop=sp)

        def normalize(avden, s, c):
            av, den = avden
            rec = recp.tile([128, 512], F32, tag="rec", name=f"rec{s}_{c}")
            nc.vector.reciprocal_approx_fast(out=rec, in_=den)
            nc.vector.tensor_mul(
                out=aoT[:, s, 512 * c:512 * (c + 1)], in0=av, in1=rec)

        def po_tile(it):
            po = psc.tile([128, 1024], F32, tag="sc", name=f"po{it}")
            for half in range(2):
                sl = slice(512 * half, 512 * (half + 1))
                for s in range(NP):
                    nc.tensor.matmul(
                        po[:, sl], aoT[:, s, 128 * it:128 * (it + 1)],
                        wo_sb[:, s, sl], start=(s == 0), stop=(s == NP - 1))
            ot = outp.tile([128, DIM], BF16, tag="out", name=f"ot{it}")
            nc.scalar.copy(out=ot, in_=po)
            nc.sync.dma_start(
                out=out_d.ap()[128 * it:128 * (it + 1), :], in_=ot)

        def unit(cur, prev, fillers):
            # interleave: scores of `cur` with AV/den of `prev`, 3-t groups;
            # normalize(prev) goes right after its last AV group so the DVE
            # reaches it before the next unit's first AV matmul needs the bank
            avden = avden_alloc(*prev) if prev else None
            fi = 0
            for g in range(0, NT, 3):
                if fillers and g in (3, 6, 9, 12) and fi < len(fillers):
                    fillers[fi]()
                    fi += 1
                for t in range(g, min(g + 3, NT)):
                    if prev:
                        avden_tile(avden, prev[0], prev[1], t)
                if prev and g + 3 >= NT:
                    normalize(avden, *prev)
                for t in range(g, min(g + 3, NT)):
                    if cur is not None:
                        sc_tile(cur[0], cur[1], t)
            while fi < len(fillers):
                fillers[fi]()
                fi += 1

        # ---- schedule: pair-major, software-pipelined by one unit ----
        proj_tile(qT, wqs[0], 0, 0)
        proj_tile(qT, wqs[0], 0, 1)
        w_fetch(1)
        proj_tile(kT, wks[0], 0, 0)
        proj_tile(kT, wks[0], 0, 1)

        seq = [(s, c) for s in range(NP) for c in range(NC)]
        prev = None
        vleft = list(range(NT // 2))
        for idx, cur in enumerate(seq):
            s, c = cur
            fillers = []
            if idx == 0:
                fillers = [(lambda j=j: v_tile(j)) for j in vleft[0:5]]
            elif idx == 1:
                fillers = [(lambda j=j: v_tile(j)) for j in vleft[5:8]]
            elif c == 2 and s < NP - 1:
                sn = s + 1
                if sn not in wqs:
                    w_fetch(sn)
                fillers = [lambda sn=sn: proj_tile(qT, wqs[sn], sn, 0),
                           lambda sn=sn: proj_tile(qT, wqs[sn], sn, 1)]
            elif c == 3 and s < NP - 1:
                sn = s + 1
                fillers = [lambda sn=sn: proj_tile(kT, wks[sn], sn, 0),
                           lambda sn=sn: proj_tile(kT, wks[sn], sn, 1)]
            elif idx >= 14:
                # out-proj chunk c-2 is complete once unit #idx-1 finished
                # (aoT(3, c-2) normalized inside unit #idx-1)
                pc = idx - 14
                fillers = [(lambda it=it: po_tile(it))
                           for it in range(4 * pc, 4 * pc + 4)]
            unit(cur, prev, fillers)
            prev = cur
        # drain av of the last unit, with out-proj chunk 2 as filler
        unit(None, prev, [(lambda it=it: po_tile(it)) for it in range(8, 12)])
        for it in range(12, 16):      # epilogue: last out-proj chunk
            po_tile(it)


def get_program():
    if "nc" not in _CACHE:
        _CACHE["nc"] = _build_program()
    return _CACHE["nc"]


def make_in_maps(x, w_qkv, w_out, b_out):
    bf = ml_dtypes.bfloat16
    w = np.ascontiguousarray(w_qkv, np.float32)
    wo = np.ascontiguousarray(w_out, np.float32)
    xTs = [np.ascontiguousarray(np.asarray(x[bb], np.float32).T).astype(bf)
           for bb in range(B)]
    in_maps = []
    for core in range(NCORES):
        bb, hg = core // 2, core % 2
        cs = slice(512 * hg, 512 * (hg + 1))
        in_maps.append({
            "xT": xTs[bb],
            "wq": np.ascontiguousarray(w[:, 0:1024][:, cs]).astype(bf),
            "wk": np.ascontiguousarray(w[:, 1024:2048][:, cs]).astype(bf),
            "wv": np.ascontiguousarray(w[:, 2048:3072][:, cs]).astype(bf),
            "wo": np.ascontiguousarray(wo[cs, :]).astype(bf),
        })
    return in_maps


def kernel(x, w_qkv, w_out, b_out):
    nc = get_program()
    in_maps = make_in_maps(x, w_qkv, w_out, b_out)
    res = bass_utils.run_bass_kernel_spmd(nc, in_maps, core_ids=list(range(NCORES)))
    bias = np.asarray(b_out, np.float32)[None, :]
    out = np.empty((B, N, DIM), np.float32)
    for bb in range(B):
        out[bb] = (res.results[2 * bb]["out"].astype(np.float32)
                   + res.results[2 * bb + 1]["out"].astype(np.float32) + bias)
    return out


# revision 28
# speedup vs baseline: 1.1883x; 1.0912x over previous
"""Trainium2 Bass kernel for multi-head attention (B=4, N=2048, DIM=1024, H=16, DH=64).

Sharding: 8 cores = 4 batches x 2 head-groups (8 heads each): column-parallel
qkv, row-parallel out-proj.  Each core computes q/k/v for its 8 heads over the
full 2048 queries, attention per head-pair with PE tile packing, and a PARTIAL
output projection; the host gather sums the two partial outputs per batch and
adds the bias.

PE schedule: software-pipelined units. Unit (s,c) interleaves, in groups of 3
j-tiles, the scores matmuls of (s,c) [(64,128) row-tiled T0/T8 pairs, 2x
concurrent] with the AV + denominator matmuls of the previous unit [(128,64)
col-tiled T0/T1, 2x concurrent; denominators via ones[128,64] matmul give a
64-row broadcast for free].  Softmax exp runs on ScalarE for 12/16 j-tiles and
on the DVE for 4/16 via a 2-pass bit-trick exp (floor extract + quadratic
mantissa correction, ~0.3% shape error, uniform scale cancels in softmax).
Normalize = reciprocal_approx_fast + one tensor_mul.  QKV projection chunks
ride as PE fillers inside units; out-proj runs as the epilogue.
"""

import numpy as np
import ml_dtypes

import concourse.bass as bass
import concourse.tile as tile
from concourse import bacc, mybir
from concourse import bass_utils

B, N, DIM = 4, 2048, 1024
HEADS, DH = 16, 64
INNER = HEADS * DH
SCALE = DH ** -0.5
NCORES = 8
HG = 8                    # heads per core
NP = HG // 2              # head pairs per core
KT = DIM // 128           # contraction tiles for projections
NT = N // 128             # 16 j tiles
NC = N // 512             # 4 i chunks
BF16 = mybir.dt.bfloat16
F32 = mybir.dt.float32
I16 = mybir.dt.int16

EXPA = float(np.float32(SCALE / np.log(2)))   # y = score * EXPA = log2(e^(s*SCALE))
EXPD = -0.34                                  # quadratic mantissa correction coeff
DVESET = (2, 6, 10, 14)                       # j-tiles exp'd on DVE (bit-trick)

_CACHE = {}


def _make_exp16():
    """Register the 2nd-pass exp op (quadratic-corrected Schraudolph ->
    bf16 bits) in the custom-DVE registry, reusing a spare opcode row."""
    from concourse.dve_spec import (
        Spec, Src0, Src1, C0, C1, C2, One, lower, _has_src1)
    from concourse.dve_uop import DveOpSpec
    from concourse import dve_ops
    from concourse.dve_ops import DveOp, get_dve_sub_opcode

    name = "CODY_WAITE_CASCADE"
    y = Src0 * C0
    f = (y - Src1) + (C2 - One)
    m = f * ((One + C1) - C1 * f)
    body = (Src1 + m) * C2

    def ref(in0, in1, s0, s1, imm2):
        yy = in0.astype(np.float32) * s0
        ff = (yy - in1) + (imm2 - 1.0)
        mm = ff * ((1.0 + s1) - s1 * ff)
        return (in1 + mm) * imm2

    spec = Spec(body=body, reference=ref)
    shas = {}
    for ver in ("v3", "v4"):
        uops = lower(spec, ver=ver)
        shas[ver] = DveOpSpec(name=name, opcode=get_dve_sub_opcode(name),
                              uops=uops, rd1_en=_has_src1(spec)).sha(ver)
    op = DveOp(name, spec, subdim=False, uops_sha=shas)
    dve_ops.OPS.append(op)
    return op


EXP16 = _make_exp16()


def _build_program():
    nc = bacc.Bacc("TRN2", target_bir_lowering=False, debug=False)
    xT_d = nc.dram_tensor("xT", [DIM, N], BF16, kind="ExternalInput")
    wq_d = nc.dram_tensor("wq", [DIM, 512], BF16, kind="ExternalInput")
    wk_d = nc.dram_tensor("wk", [DIM, 512], BF16, kind="ExternalInput")
    wv_d = nc.dram_tensor("wv", [DIM, 512], BF16, kind="ExternalInput")
    wo_d = nc.dram_tensor("wo", [512, DIM], BF16, kind="ExternalInput")
    out_d = nc.dram_tensor("out", [N, DIM], BF16, kind="ExternalOutput")
    with tile.TileContext(nc) as tc:
        _emit(tc, nc, xT_d, wq_d, wk_d, wv_d, wo_d, out_d)
    nc.compile()
    return nc


def _emit(tc, nc, xT_d, wq_d, wk_d, wv_d, wo_d, out_d):
    from contextlib import ExitStack

    xT_r = xT_d.ap().rearrange("(t p) n -> p t n", p=128)    # [128, 8, 2048]
    wq_r = wq_d.ap().rearrange("(t p) e -> p t e", p=128)    # [128, 8, 512]
    wk_r = wk_d.ap().rearrange("(t p) e -> p t e", p=128)
    wv_r = wv_d.ap().rearrange("(t p) e -> p t e", p=128)
    wo_r = wo_d.ap().rearrange("(t p) d -> p t d", p=128)    # [128, 4, 1024]

    with ExitStack() as ctx:
        consts = ctx.enter_context(tc.tile_pool(name="consts", bufs=1))
        qkv = ctx.enter_context(tc.tile_pool(name="qkv", bufs=1))
        atp = ctx.enter_context(tc.tile_pool(name="atp", bufs=16))
        up = ctx.enter_context(tc.tile_pool(name="up", bufs=4))
        recp = ctx.enter_context(tc.tile_pool(name="recp", bufs=2))
        outp = ctx.enter_context(tc.tile_pool(name="outp", bufs=2))
        xp = ctx.enter_context(tc.tile_pool(name="xp", bufs=1))
        wp = ctx.enter_context(tc.tile_pool(name="wp", bufs=1))
        wqk = ctx.enter_context(tc.tile_pool(name="wqk", bufs=4))
        psc = ctx.enter_context(tc.tile_pool(name="psc", bufs=3, space="PSUM"))
        pwk = ctx.enter_context(tc.tile_pool(name="pwk", bufs=2, space="PSUM"))

        # ---- input DMAs, spread over 3 queues; first-needed first ----
        wqs = {0: wqk.tile([128, KT, 128], BF16, tag="w", name="wq0")}
        wks = {0: wqk.tile([128, KT, 128], BF16, tag="w", name="wk0")}
        nc.sync.dma_start(out=wqs[0], in_=wq_r[:, :, 0:128])
        nc.scalar.dma_start(out=wks[0], in_=wk_r[:, :, 0:128])
        xTk = xp.tile([128, KT, N], BF16)
        qs = [nc.sync, nc.gpsimd, nc.scalar]
        for k in range(KT):
            qs[k % 3].dma_start(out=xTk[:, k, :], in_=xT_r[:, k, :])
        wv_sb = wp.tile([128, KT, 512], BF16)
        nc.gpsimd.dma_start(out=wv_sb, in_=wv_r)
        wo_sb = consts.tile([128, 4, DIM], BF16)
        nc.gpsimd.dma_start(out=wo_sb, in_=wo_r)
        ones = consts.tile([128, 64], BF16)
        nc.vector.memset(ones, 1.0)

        # ---- persistent sbuf ----
        qT = qkv.tile([128, NP, N], BF16)     # pair s rows: head 2s | 2s+1
        kT = qkv.tile([128, NP, N], BF16)
        v = qkv.tile([128, NT, HG, DH], BF16)
        aoT = qkv.tile([128, NP, N], BF16)

        # ---- emitters ----
        def w_fetch(s):
            wqs[s] = wqk.tile([128, KT, 128], BF16, tag="w", name=f"wq{s}")
            wks[s] = wqk.tile([128, KT, 128], BF16, tag="w", name=f"wk{s}")
            nc.sync.dma_start(out=wqs[s], in_=wq_r[:, :, 128 * s:128 * (s + 1)])
            nc.sync.dma_start(out=wks[s], in_=wk_r[:, :, 128 * s:128 * (s + 1)])

        def proj_tile(dst, w_sb, s, ih):
            # [128 e, 1024 i] chunk of q or k pair-slice s
            ps = psc.tile([128, 1024], F32, tag="sc", name=f"pj{dst is kT}_{s}_{ih}")
            for half in range(2):
                sl = slice(512 * half, 512 * (half + 1))
                for k in range(KT):
                    nc.tensor.matmul(
                        ps[:, sl], w_sb[:, k, :],
                        xTk[:, k, 1024 * ih + 512 * half:
                            1024 * ih + 512 * (half + 1)],
                        start=(k == 0), stop=(k == KT - 1))
            nc.vector.tensor_copy(
                out=dst[:, s, 1024 * ih:1024 * (ih + 1)], in_=ps)

        def v_tile(j):
            # v j-tile pair (2j, 2j+1) for all 8 heads
            ps = psc.tile([128, 1024], F32, tag="sc", name=f"v{j}")
            for tt in range(2):
                for k in range(KT):
                    nc.tensor.matmul(
                        ps[:, 512 * tt:512 * (tt + 1)],
                        xTk[:, k, 128 * (2 * j + tt):128 * (2 * j + tt + 1)],
                        wv_sb[:, k, :], start=(k == 0), stop=(k == KT - 1))
            nc.vector.tensor_copy(
                out=v[:, 2 * j:2 * j + 2, :, :],
                in_=ps.rearrange("p (t2 h d) -> p t2 h d", t2=2, h=HG))

        ats = {}

        def sc_tile(s, c, t):
            # scores^T pair s, i-chunk c, j-tile t + exp dispatch
            isl = slice(512 * c, 512 * (c + 1))
            sc = psc.tile([128, 1024], F32, tag="sc", name=f"sc{s}_{c}_{t}")
            nc.tensor.matmul(
                sc[:, 0:512], kT[0:64, s, 128 * t:128 * (t + 1)],
                qT[0:64, s, isl], start=True, stop=True)
            nc.tensor.matmul(
                sc[:, 512:1024], kT[64:128, s, 128 * t:128 * (t + 1)],
                qT[64:128, s, isl], start=True, stop=True)
            at = atp.tile([128, 1024], BF16, tag="at", name=f"at{s}_{c}_{t}")
            ats[(s, c, t)] = at
            if t in DVESET:
                u = up.tile([128, 1024], I16, tag="u", name=f"u{s}_{c}_{t}")
                for hf in range(2):
                    sl = slice(512 * hf, 512 * (hf + 1))
                    nc.vector.tensor_scalar(
                        out=u[:, sl], in0=sc[:, sl], scalar1=EXPA,
                        scalar2=126.5, op0=mybir.AluOpType.mult,
                        op1=mybir.AluOpType.add)
                    nc.vector._custom_dve(
                        EXP16, out=at.bitcast(I16)[:, sl], in0=sc[:, sl],
                        in1=u[:, sl], s0=EXPA, s1=EXPD, imm2=128.0)
            else:
                nc.scalar.activation(
                    out=at, in_=sc,
                    func=mybir.ActivationFunctionType.Exp, scale=SCALE)

        def avden_alloc(s, c):
            av = pwk.tile([128, 512], F32, tag="wk", name=f"av{s}_{c}")
            den = pwk.tile([128, 512], F32, tag="wk", name=f"den{s}_{c}")
            return av, den

        def avden_tile(avden, s, c, t):
            av, den = avden
            at = ats.pop((s, c, t))
            st, sp = (t == 0), (t == NT - 1)
            a0 = at[:, 0:512].bitcast(BF16) if t in DVESET else at[:, 0:512]
            a1 = at[:, 512:1024].bitcast(BF16) if t in DVESET else at[:, 512:1024]
            nc.tensor.matmul(av[0:64, :], v[:, t, 2 * s, :], a0,
                             start=st, stop=sp)
            nc.tensor.matmul(av[64:128, :], v[:, t, 2 * s + 1, :], a1,
                             start=st, stop=sp)
            nc.tensor.matmul(den[0:64, :], ones, a0, start=st, stop=sp)
            nc.tensor.matmul(den[64:128, :], ones, a1, start=st, stop=sp)

        def normalize(avden, s, c):
            av, den = avden
            rec = recp.tile([128, 512], F32, tag="rec", name=f"rec{s}_{c}")
            nc.vector.reciprocal_approx_fast(out=rec, in_=den)
            nc.vector.tensor_mul(
                out=aoT[:, s, 512 * c:512 * (c + 1)], in0=av, in1=rec)

        def po_tile(it):
            po = psc.tile([128, 1024], F32, tag="sc", name=f"po{it}")
            for half in range(2):
                sl = slice(512 * half, 512 * (half + 1))
                for s in range(NP):
                    nc.tensor.matmul(
                        po[:, sl], aoT[:, s, 128 * it:128 * (it + 1)],
                        wo_sb[:, s, sl], start=(s == 0), stop=(s == NP - 1))
            ot = outp.tile([128, DIM], BF16, tag="out", name=f"ot{it}")
            nc.scalar.copy(out=ot, in_=po)
            nc.sync.dma_start(
                out=out_d.ap()[128 * it:128 * (it + 1), :], in_=ot)

        def unit(cur, prev, fillers):
            # interleave: scores of `cur` with AV/den of `prev`, 3-t groups;
            # normalize(prev) goes right after its last AV group so the DVE
            # reaches it before the next unit's first AV matmul needs the bank
            avden = avden_alloc(*prev) if prev else None
            fi = 0
            for g in range(0, NT, 4):
                if fillers and g in (4, 8, 12) and fi < len(fillers):
                    fillers[fi]()
                    fi += 1
                for t in range(g, min(g + 4, NT)):
                    if prev:
                        avden_tile(avden, prev[0], prev[1], t)
                if prev and g + 4 >= NT:
                    normalize(avden, *prev)
                for t in range(g, min(g + 4, NT)):
                    if cur is not None:
                        sc_tile(cur[0], cur[1], t)
            while fi < len(fillers):
                fillers[fi]()
                fi += 1

        # ---- schedule: pair-major, software-pipelined by one unit ----
        proj_tile(qT, wqs[0], 0, 0)
        proj_tile(qT, wqs[0], 0, 1)
        w_fetch(1)
        proj_tile(kT, wks[0], 0, 0)
        proj_tile(kT, wks[0], 0, 1)

        seq = [(s, c) for s in range(NP) for c in range(NC)]
        prev = None
        vleft = list(range(NT // 2))
        for idx, cur in enumerate(seq):
            s, c = cur
            fillers = []
            if idx == 0:
                fillers = [(lambda j=j: v_tile(j)) for j in vleft[0:5]]
            elif idx == 1:
                fillers = [(lambda j=j: v_tile(j)) for j in vleft[5:8]]
            elif c == 2 and s < NP - 1:
                sn = s + 1
                if sn not in wqs:
                    w_fetch(sn)
                fillers = [lambda sn=sn: proj_tile(qT, wqs[sn], sn, 0),
                           lambda sn=sn: proj_tile(qT, wqs[sn], sn, 1)]
            elif c == 3 and s < NP - 1:
                sn = s + 1
                fillers = [lambda sn=sn: proj_tile(kT, wks[sn], sn, 0),
                           lambda sn=sn: proj_tile(kT, wks[sn], sn, 1)]
            elif idx >= 14:
                # out-proj chunk c-2 is complete once unit #idx-1 finished
                # (aoT(3, c-2) normalized inside unit #idx-1)
                pc = idx - 14
                fillers = [(lambda it=it: po_tile(it))
                           for it in range(4 * pc, 4 * pc + 4)]
            unit(cur, prev, fillers)
            prev = cur
        # drain av of the last unit, with out-proj chunk 2 as filler
        unit(None, prev, [(lambda it=it: po_tile(it)) for it in range(8, 12)])
        for it in range(12, 16):      # epilogue: last out-proj chunk
            po_tile(it)


def get_program():
    if "nc" not in _CACHE:
        _CACHE["nc"] = _build_program()
    return _CACHE["nc"]


def make_in_maps(x, w_qkv, w_out, b_out):
    bf = ml_dtypes.bfloat16
    w = np.ascontiguousarray(w_qkv, np.float32)
    wo = np.ascontiguousarray(w_out, np.float32)
    xTs = [np.ascontiguousarray(np.asarray(x[bb], np.float32).T).astype(bf)
           for bb in range(B)]
    in_maps = []
    for core in range(NCORES):
        bb, hg = core // 2, core % 2
        cs = slice(512 * hg, 512 * (hg + 1))
        in_maps.append({
            "xT": xTs[bb],
            "wq": np.ascontiguousarray(w[:, 0:1024][:, cs]).astype(bf),
            "wk": np.ascontiguousarray(w[:, 1024:2048][:, cs]).astype(bf),
            "wv": np.ascontiguousarray(w[:, 2048:3072][:, cs]).astype(bf),
            "wo": np.ascontiguousarray(wo[cs, :]).astype(bf),
        })
    return in_maps


def kernel(x, w_qkv, w_out, b_out):
    nc = get_program()
    in_maps = make_in_maps(x, w_qkv, w_out, b_out)
    res = bass_utils.run_bass_kernel_spmd(nc, in_maps, core_ids=list(range(NCORES)))
    bias = np.asarray(b_out, np.float32)[None, :]
    out = np.empty((B, N, DIM), np.float32)
    for bb in range(B):
        out[bb] = (res.results[2 * bb]["out"].astype(np.float32)
                   + res.results[2 * bb + 1]["out"].astype(np.float32) + bias)
    return out
